# revision 29
# baseline (speedup 1.0000x reference)
"""MoE gate kernel for TRN2: logits = x @ w, top-8 over 64 experts, softmax.

Sharding: x [65536, 1024] split by token across 8 cores (8192 tokens each),
fed pre-transposed + pre-tiled. w [1024, 64] replicated.

Precision: x shipped as exact fp16 hi/lo pair (x == xh + xl + O(2^-22)),
w split into fp16 hi/lo on device (same DVE conversions as the proven
kernel, so wh/wl are bit-identical to it). The two w halves are packed
side by side into one [128, 128] stationary [wh | wl], so ONE moving
pass of xh computes xh@wh (PSUM partitions 0-63, "B") AND xh@wl
(partitions 64-127, "W") simultaneously; the xl pass adds xl@wh / xl@wl.
16 matmuls/chunk instead of the 3-pass kernel's 24 — B's partial order
(xh@wh k0..7, xl@wh k0..7) is bit-identical to that kernel's first two
passes, and W accumulates at magnitude ~2^-11 so it is error-free.

Top-8 robustness: a plain fl(B+W) comparison carries +-ulp(34) ~ 2e-6
noise, which mis-orders a sub-ULP logit pair in the eval set.  Instead
the logit is kept as an exact Dekker pair: s = fl(B+W), err = W-(s-B)
(TwoSum; s+err == B+W exactly), and the selection key is
r = fl(fl(s - m) + err) with m = per-token max(s).  s-m is exact for
all top-8 contenders (Sterbenz: s >= m/2), so r orders contenders with
error <= ulp(~10)/2 ~ 2.4e-7, an order of magnitude below every
remaining decision margin.  Softmax is shift-invariant, so feeding it
the r-values directly yields the same scores.

DMA: only sync/scalar/gpsimd can issue DMAs, and each queue tops out
around 125-150 GB/s, so the 33.5MB of x per core is striped across all
three, weighted 3:3:2 (gpsimd's software-dynamic queue is the slowest).
Host pre-packs x as [chunk][half][128p][4k][512t] so each 512KB half-
chunk start is contiguous in DRAM; the first two chunks go as 256KB
quarters to shorten the ramp.  The per-chunk loop is software-pipelined
(issue loads for chunk c, compute chunk c-4) so compute ops never block
later x-load issues in an engine's instruction stream; per-chunk output
DMAs are lagged one chunk for the same reason.
"""

import sys

sys.path.insert(0, "/opt/trn_rl_repo")

from contextlib import ExitStack

import numpy as np

import concourse.bacc as bacc
import concourse.mybir as mybir
import concourse.tile as tile
from concourse import masks
from concourse.bass_utils import run_bass_kernel_spmd

N_CORES = 8
TOKENS = 65536
D = 1024
E = 64
TOPK = 8
TOK_PER_CORE = TOKENS // N_CORES
CHUNK = 512  # tokens per chunk (PSUM bank = 512 f32)
SUBS = CHUNK // 128
KCH = D // 128  # contraction chunks
N_CHUNK = TOK_PER_CORE // CHUNK
HKCH = KCH // 2  # k-chunks per DMA half
DELTA = 4  # software pipeline depth (chunks)

F32 = mybir.dt.float32
F16 = mybir.dt.float16
U32 = mybir.dt.uint32


def build_program(tok_per_core=TOK_PER_CORE):
    n_chunk = tok_per_core // CHUNK
    nc = bacc.Bacc(
        "TRN2", target_bir_lowering=False, debug=False, num_devices=N_CORES
    )
    xh_d = nc.dram_tensor(
        "xh", [n_chunk, 2, 128, HKCH * CHUNK], F16, kind="ExternalInput"
    ).ap()
    xl_d = nc.dram_tensor(
        "xl", [n_chunk, 2, 128, HKCH * CHUNK], F16, kind="ExternalInput"
    ).ap()
    w_d = nc.dram_tensor("w", [D, E], F32, kind="ExternalInput").ap()
    scores_d = nc.dram_tensor(
        "scores", [n_chunk, 128, SUBS, TOPK], F32, kind="ExternalOutput"
    ).ap()
    experts_d = nc.dram_tensor(
        "experts", [n_chunk, 128, SUBS, TOPK], U32, kind="ExternalOutput"
    ).ap()

    with tile.TileContext(nc) as tc, ExitStack() as ctx:
        wpool = ctx.enter_context(tc.tile_pool(name="wpool", bufs=1))
        xpool = ctx.enter_context(tc.tile_pool(name="xpool", bufs=8))
        ltpool = ctx.enter_context(tc.tile_pool(name="ltpool", bufs=3))
        ptpool = ctx.enter_context(tc.tile_pool(name="ptpool", bufs=3, space="PSUM"))
        ppool = ctx.enter_context(tc.tile_pool(name="ppool", bufs=3, space="PSUM"))
        spool = ctx.enter_context(tc.tile_pool(name="spool", bufs=4))

        ident = wpool.tile([128, 128], F32)
        masks.make_identity(nc, ident[:])

        # w split on DEVICE with the same DVE fp32->fp16 conversions as
        # the proven kernel; packed side by side: whl = [wh | wl]
        w_t = wpool.tile([128, KCH, E], F32)
        nc.gpsimd.dma_start(
            out=w_t[:], in_=w_d.rearrange("(k p) e -> p k e", p=128)
        )
        whl = wpool.tile([128, KCH, 2 * E], F16)
        nc.vector.tensor_copy(whl[:, :, 0:E], w_t[:])
        wl32 = wpool.tile([128, KCH, E], F32)
        nc.vector.tensor_sub(wl32[:], w_t[:], whl[:, :, 0:E])
        nc.vector.tensor_copy(whl[:, :, E : 2 * E], wl32[:])

        # weighted striping: per 8 pieces, 3 to sync, 3 to scalar, 2 to
        # gpsimd (its software-dynamic queue is ~20% slower, and this
        # pattern never hands gpsimd two pieces of the same chunk — an
        # equal i%3 rotation does, spiking chunk latency and stalling
        # the PE)
        qengines = [nc.sync, nc.scalar, nc.gpsimd]
        qorder = [0, 1, 2, 0, 1, 0, 1, 2]
        piece_i = [0]

        def next_q():
            q = qengines[qorder[piece_i[0] % 8]]
            piece_i[0] += 1
            return q

        xtiles = {}

        def issue_loads(c):
            xh_t = xpool.tile([128, KCH, CHUNK], F16, tag="xh_t")
            xl_t = xpool.tile([128, KCH, CHUNK], F16, tag="xl_t")
            xtiles[c] = (xh_t, xl_t)
            halves = [
                (xh_t[:, 0:HKCH, :], xh_d[c, 0]),
                (xh_t[:, HKCH:KCH, :], xh_d[c, 1]),
                (xl_t[:, 0:HKCH, :], xl_d[c, 0]),
                (xl_t[:, HKCH:KCH, :], xl_d[c, 1]),
            ]
            # 256KB quarter pieces throughout: per-dma_start queue
            # turnaround gaps throttle the HW queues to ~120 GB/s with
            # 512KB starts, while 256KB starts sustain ~150+ GB/s (and
            # the weighted qorder still caps gpsimd at 512KB per chunk)
            Q = HKCH * CHUNK // 2
            for dst, src in halves:
                next_q().dma_start(out=dst[:, 0 : HKCH // 2, :], in_=src[:, 0:Q])
                next_q().dma_start(out=dst[:, HKCH // 2 : HKCH, :], in_=src[:, Q:])

        pend = {}

        def flush_out(c):
            sc4, idx4 = pend.pop(c)
            nc.gpsimd.dma_start(out=scores_d[c], in_=sc4[:])
            nc.gpsimd.dma_start(out=experts_d[c], in_=idx4[:])

        def compute(c):
            xh_t, xl_t = xtiles.pop(c)

            # 16-matmul chain; partitions 0-63 see xh@wh k0..7 then
            # xl@wh k0..7 — bit-identical to the proven kernel's first
            # 16 partials
            lps = ptpool.tile([128, CHUNK], F32)
            for k in range(KCH):
                nc.tensor.matmul(
                    lps[:], whl[:, k, :], xh_t[:, k, :],
                    start=(k == 0), stop=False,
                )
            for k in range(KCH):
                nc.tensor.matmul(
                    lps[:], whl[:, k, :], xl_t[:, k, :],
                    start=False, stop=(k == KCH - 1),
                )

            # PSUM -> SBUF on the scalar engine (frees DVE)
            ltile = ltpool.tile([128, CHUNK], F32)
            nc.scalar.activation(
                ltile[:], lps[:], mybir.ActivationFunctionType.Copy
            )

            # 4 transposes into one PSUM bank -> [tok, s, B-col | W-col]
            tpb = ppool.tile([128, SUBS, 128], F32)
            for s in range(SUBS):
                nc.tensor.transpose(
                    tpb[:, s, :], ltile[:, s * 128 : (s + 1) * 128], ident[:]
                )
            # stage the W half through SBUF (ops may read only one PSUM
            # operand, and gpsimd none)
            wlp4 = spool.tile([128, SUBS, E], F32, tag="wlp4")
            nc.scalar.activation(
                wlp4[:], tpb[:, :, E : 2 * E],
                mybir.ActivationFunctionType.Copy,
            )

            # Dekker TwoSum: s4 + er4 == B + W exactly
            s4 = spool.tile([128, SUBS, E], F32, tag="s4")
            nc.vector.tensor_add(s4[:], tpb[:, :, 0:E], wlp4[:])
            bb4 = spool.tile([128, SUBS, E], F32, tag="bb4")
            nc.vector.tensor_sub(bb4[:], s4[:], tpb[:, :, 0:E])
            er4 = spool.tile([128, SUBS, E], F32, tag="er4")
            nc.vector.tensor_sub(er4[:], wlp4[:], bb4[:])
            # refined selection key r = (s - max) + err (exact shift for
            # contenders, then one rounding at ~ulp(10))
            m4 = spool.tile([128, SUBS, 1], F32, tag="m4")
            nc.vector.tensor_reduce(
                m4[:, :, 0], s4[:], mybir.AxisListType.X, mybir.AluOpType.max
            )
            d4 = spool.tile([128, SUBS, E], F32, tag="d4")
            nc.vector.tensor_sub(
                d4[:], s4[:], m4[:].broadcast_to((128, SUBS, E))
            )
            r4 = spool.tile([128, SUBS, E], F32, tag="r4")
            nc.vector.tensor_add(r4[:], d4[:], er4[:])

            vals4 = spool.tile([128, SUBS, TOPK], F32, tag="vals4")
            idx4 = spool.tile([128, SUBS, TOPK], U32, tag="idx4")
            for s in range(SUBS):
                nc.vector.max(vals4[:, s, :], r4[:, s, :])
                nc.vector.max_index(idx4[:, s, :], vals4[:, s, :], r4[:, s, :])

            # batched softmax over the 4 sub-tiles: exp(v - max) / sum
            # (r is the logit shifted per token, so scores are unchanged)
            sh4 = spool.tile([128, SUBS, TOPK], F32, tag="sh4")
            nc.vector.tensor_sub(
                sh4[:],
                vals4[:],
                vals4[:, :, 0:1].broadcast_to((128, SUBS, TOPK)),
            )
            ex4 = spool.tile([128, SUBS, TOPK], F32, tag="ex4")
            nc.scalar.activation(
                ex4[:], sh4[:], mybir.ActivationFunctionType.Exp
            )
            sums4 = spool.tile([128, SUBS, 1], F32, tag="sums4")
            nc.vector.tensor_reduce(
                sums4[:, :, 0],
                ex4[:],
                mybir.AxisListType.X,
                mybir.AluOpType.add,
            )
            rs4 = spool.tile([128, SUBS, 1], F32, tag="rs4")
            nc.vector.reciprocal(rs4[:], sums4[:])
            sc4 = spool.tile([128, SUBS, TOPK], F32, tag="sc4")
            nc.vector.tensor_mul(
                sc4[:], ex4[:], rs4[:].broadcast_to((128, SUBS, TOPK))
            )
            pend[c] = (sc4, idx4)

        for step in range(n_chunk + DELTA):
            if step < n_chunk:
                issue_loads(step)
            if step >= DELTA:
                c = step - DELTA
                compute(c)
                # output DMAs lag one chunk so they never gate the next
                # x-load issue in gpsimd's stream
                if c > 0:
                    flush_out(c - 1)
        flush_out(n_chunk - 1)

    nc.compile()
    return nc


_PROGRAM = None


def _get_program():
    global _PROGRAM
    if _PROGRAM is None:
        _PROGRAM = build_program()
    return _PROGRAM


def _make_in_maps(x, weights):
    x = np.asarray(x, dtype=np.float32)
    w = np.asarray(weights, dtype=np.float32)
    maps = []
    for i in range(N_CORES):
        xs = np.ascontiguousarray(x[i * TOK_PER_CORE : (i + 1) * TOK_PER_CORE].T)
        xh = xs.astype(np.float16)
        xl = (xs - xh.astype(np.float32)).astype(np.float16)

        def pack(a):
            # [1024, 8192] -> [chunk, half, 128p, 4k, 512t] contiguous
            return np.ascontiguousarray(
                a.reshape(2, HKCH, 128, N_CHUNK, CHUNK).transpose(3, 0, 2, 1, 4)
            ).reshape(N_CHUNK, 2, 128, HKCH * CHUNK)

        maps.append({"xh": pack(xh), "xl": pack(xl), "w": w})
    return maps


def run(x, weights, trace=False):
    nc = _get_program()
    res = run_bass_kernel_spmd(
        nc, _make_in_maps(x, weights), list(range(N_CORES)), trace=trace
    )

    def unblock(a):
        # [n_chunk, 128, SUBS, TOPK] -> [tok_per_core, TOPK]
        return np.ascontiguousarray(a.transpose(0, 2, 1, 3)).reshape(-1, TOPK)

    scores = np.concatenate(
        [unblock(res.results[i]["scores"]) for i in range(N_CORES)]
    )
    experts = np.concatenate(
        [unblock(res.results[i]["experts"]).astype(np.int32) for i in range(N_CORES)]
    )
    return (scores, experts), res


def kernel(x, weights):
    out, _ = run(x, weights)
    return out


# revision 30
# speedup vs baseline: 1.0737x; 1.0737x over previous
"""MoE gate kernel for TRN2: logits = x @ w, top-8 over 64 experts, softmax.

Sharding: x [65536, 1024] split by token across 8 cores (8192 tokens each),
fed pre-transposed + pre-tiled. w [1024, 64] replicated.

Precision: x shipped as exact fp16 hi/lo pair (x == xh + xl + O(2^-22)),
w split into fp16 hi/lo on device (same DVE conversions as the proven
kernel, so wh/wl are bit-identical to it). The two w halves are packed
side by side into one [128, 128] stationary [wh | wl], so ONE moving
pass of xh computes xh@wh (PSUM partitions 0-63, "B") AND xh@wl
(partitions 64-127, "W") simultaneously; the xl pass adds xl@wh / xl@wl.
16 matmuls/chunk instead of the 3-pass kernel's 24 — B's partial order
(xh@wh k0..7, xl@wh k0..7) is bit-identical to that kernel's first two
passes, and W accumulates at magnitude ~2^-11 so it is error-free.

Top-8 robustness: a plain fl(B+W) comparison carries +-ulp(34) ~ 2e-6
noise, which mis-orders a sub-ULP logit pair in the eval set.  Instead
the logit is kept as an exact Dekker pair: s = fl(B+W), err = W-(s-B)
(TwoSum; s+err == B+W exactly), and the selection key is
r = fl(fl(s - m) + err) with m = per-token max(s).  s-m is exact for
all top-8 contenders (Sterbenz: s >= m/2), so r orders contenders with
error <= ulp(~10)/2 ~ 2.4e-7, an order of magnitude below every
remaining decision margin.  Softmax is shift-invariant, so feeding it
the r-values directly yields the same scores.

DMA: only sync/scalar/gpsimd can issue DMAs, and each queue tops out
around 125-150 GB/s, so the 33.5MB of x per core is striped across all
three, weighted 3:3:2 (gpsimd's software-dynamic queue is the slowest).
Host pre-packs x as [chunk][half][128p][4k][512t] so each 512KB half-
chunk start is contiguous in DRAM; the first two chunks go as 256KB
quarters to shorten the ramp.  The per-chunk loop is software-pipelined
(issue loads for chunk c, compute chunk c-4) so compute ops never block
later x-load issues in an engine's instruction stream; per-chunk output
DMAs are lagged one chunk for the same reason.
"""

import sys

sys.path.insert(0, "/opt/trn_rl_repo")

from contextlib import ExitStack

import numpy as np

import concourse.bacc as bacc
import concourse.mybir as mybir
import concourse.tile as tile
from concourse import masks
from concourse.bass_utils import run_bass_kernel_spmd

N_CORES = 8
TOKENS = 65536
D = 1024
E = 64
TOPK = 8
TOK_PER_CORE = TOKENS // N_CORES
CHUNK = 512  # tokens per chunk (PSUM bank = 512 f32)
SUBS = CHUNK // 128
KCH = D // 128  # contraction chunks
N_CHUNK = TOK_PER_CORE // CHUNK
HKCH = KCH // 2  # k-chunks per DMA half
DELTA = 4  # software pipeline depth (chunks)

F32 = mybir.dt.float32
F16 = mybir.dt.float16
U32 = mybir.dt.uint32


def build_program(tok_per_core=TOK_PER_CORE):
    n_chunk = tok_per_core // CHUNK
    nc = bacc.Bacc(
        "TRN2", target_bir_lowering=False, debug=False, num_devices=N_CORES
    )
    xh_d = nc.dram_tensor(
        "xh", [n_chunk, 2, 128, HKCH * CHUNK], F16, kind="ExternalInput"
    ).ap()
    xl_d = nc.dram_tensor(
        "xl", [n_chunk, 2, 128, HKCH * CHUNK], F16, kind="ExternalInput"
    ).ap()
    w_d = nc.dram_tensor("w", [D, E], F32, kind="ExternalInput").ap()
    scores_d = nc.dram_tensor(
        "scores", [n_chunk, 128, SUBS, TOPK], F32, kind="ExternalOutput"
    ).ap()
    experts_d = nc.dram_tensor(
        "experts", [n_chunk, 128, SUBS, TOPK], U32, kind="ExternalOutput"
    ).ap()

    with tile.TileContext(nc) as tc, ExitStack() as ctx:
        wpool = ctx.enter_context(tc.tile_pool(name="wpool", bufs=1))
        xpool = ctx.enter_context(tc.tile_pool(name="xpool", bufs=8))
        ltpool = ctx.enter_context(tc.tile_pool(name="ltpool", bufs=3))
        ptpool = ctx.enter_context(tc.tile_pool(name="ptpool", bufs=3, space="PSUM"))
        ppool = ctx.enter_context(tc.tile_pool(name="ppool", bufs=3, space="PSUM"))
        spool = ctx.enter_context(tc.tile_pool(name="spool", bufs=4))

        ident = wpool.tile([128, 128], F32)
        masks.make_identity(nc, ident[:])

        # w split on DEVICE with the same DVE fp32->fp16 conversions as
        # the proven kernel; packed side by side: whl = [wh | wl]
        w_t = wpool.tile([128, KCH, E], F32)
        nc.gpsimd.dma_start(
            out=w_t[:], in_=w_d.rearrange("(k p) e -> p k e", p=128)
        )
        whl = wpool.tile([128, KCH, 2 * E], F16)
        nc.vector.tensor_copy(whl[:, :, 0:E], w_t[:])
        wl32 = wpool.tile([128, KCH, E], F32)
        nc.vector.tensor_sub(wl32[:], w_t[:], whl[:, :, 0:E])
        nc.vector.tensor_copy(whl[:, :, E : 2 * E], wl32[:])

        # weighted striping: per 8 pieces, 3 to sync, 3 to scalar, 2 to
        # gpsimd (its software-dynamic queue is ~20% slower, and this
        # pattern never hands gpsimd two pieces of the same chunk — an
        # equal i%3 rotation does, spiking chunk latency and stalling
        # the PE)
        qengines = [nc.sync, nc.scalar, nc.gpsimd]
        qorder = [0, 1, 2, 0, 1, 0, 1, 2]
        piece_i = [0]

        def next_q():
            q = qengines[qorder[piece_i[0] % 8]]
            piece_i[0] += 1
            return q

        xtiles = {}

        def issue_loads(c):
            xh_t = xpool.tile([128, KCH, CHUNK], F16, tag="xh_t")
            xl_t = xpool.tile([128, KCH, CHUNK], F16, tag="xl_t")
            xtiles[c] = (xh_t, xl_t)
            halves = [
                (xh_t[:, 0:HKCH, :], xh_d[c, 0]),
                (xh_t[:, HKCH:KCH, :], xh_d[c, 1]),
                (xl_t[:, 0:HKCH, :], xl_d[c, 0]),
                (xl_t[:, HKCH:KCH, :], xl_d[c, 1]),
            ]
            if c < 2:
                # ramp: quarter-size pieces spread the first chunks across
                # all three queues (256KB pieces for ALL chunks were tried
                # and regress ~5us: the extra issue overhead outweighs any
                # per-queue rate gain)
                Q = HKCH * CHUNK // 2
                for dst, src in halves:
                    next_q().dma_start(out=dst[:, 0 : HKCH // 2, :], in_=src[:, 0:Q])
                    next_q().dma_start(out=dst[:, HKCH // 2 : HKCH, :], in_=src[:, Q:])
            else:
                for dst, src in halves:
                    next_q().dma_start(out=dst, in_=src)

        pend = {}

        def flush_out(c):
            sc4, idx4 = pend.pop(c)
            nc.gpsimd.dma_start(out=scores_d[c], in_=sc4[:])
            nc.gpsimd.dma_start(out=experts_d[c], in_=idx4[:])

        def compute(c):
            xh_t, xl_t = xtiles.pop(c)

            # 16-matmul chain; partitions 0-63 see xh@wh k0..7 then
            # xl@wh k0..7 — bit-identical to the proven kernel's first
            # 16 partials
            lps = ptpool.tile([128, CHUNK], F32)
            for k in range(KCH):
                nc.tensor.matmul(
                    lps[:], whl[:, k, :], xh_t[:, k, :],
                    start=(k == 0), stop=False,
                )
            for k in range(KCH):
                nc.tensor.matmul(
                    lps[:], whl[:, k, :], xl_t[:, k, :],
                    start=False, stop=(k == KCH - 1),
                )

            # PSUM -> SBUF on the scalar engine (frees DVE)
            ltile = ltpool.tile([128, CHUNK], F32)
            nc.scalar.activation(
                ltile[:], lps[:], mybir.ActivationFunctionType.Copy
            )

            # 4 transposes into one PSUM bank -> [tok, s, B-col | W-col]
            tpb = ppool.tile([128, SUBS, 128], F32)
            for s in range(SUBS):
                nc.tensor.transpose(
                    tpb[:, s, :], ltile[:, s * 128 : (s + 1) * 128], ident[:]
                )
            # stage the W half through SBUF (ops may read only one PSUM
            # operand, and gpsimd none)
            wlp4 = spool.tile([128, SUBS, E], F32, tag="wlp4")
            nc.scalar.activation(
                wlp4[:], tpb[:, :, E : 2 * E],
                mybir.ActivationFunctionType.Copy,
            )

            # Dekker TwoSum: s4 + er4 == B + W exactly
            s4 = spool.tile([128, SUBS, E], F32, tag="s4")
            nc.vector.tensor_add(s4[:], tpb[:, :, 0:E], wlp4[:])
            bb4 = spool.tile([128, SUBS, E], F32, tag="bb4")
            nc.vector.tensor_sub(bb4[:], s4[:], tpb[:, :, 0:E])
            er4 = spool.tile([128, SUBS, E], F32, tag="er4")
            nc.vector.tensor_sub(er4[:], wlp4[:], bb4[:])
            # refined selection key r = (s - max) + err (exact shift for
            # contenders, then one rounding at ~ulp(10))
            m4 = spool.tile([128, SUBS, 1], F32, tag="m4")
            nc.vector.tensor_reduce(
                m4[:, :, 0], s4[:], mybir.AxisListType.X, mybir.AluOpType.max
            )
            d4 = spool.tile([128, SUBS, E], F32, tag="d4")
            nc.vector.tensor_sub(
                d4[:], s4[:], m4[:].broadcast_to((128, SUBS, E))
            )
            r4 = spool.tile([128, SUBS, E], F32, tag="r4")
            nc.vector.tensor_add(r4[:], d4[:], er4[:])

            vals4 = spool.tile([128, SUBS, TOPK], F32, tag="vals4")
            idx4 = spool.tile([128, SUBS, TOPK], U32, tag="idx4")
            for s in range(SUBS):
                nc.vector.max(vals4[:, s, :], r4[:, s, :])
                nc.vector.max_index(idx4[:, s, :], vals4[:, s, :], r4[:, s, :])

            # batched softmax over the 4 sub-tiles: exp(v - max) / sum
            # (r is the logit shifted per token, so scores are unchanged)
            sh4 = spool.tile([128, SUBS, TOPK], F32, tag="sh4")
            nc.vector.tensor_sub(
                sh4[:],
                vals4[:],
                vals4[:, :, 0:1].broadcast_to((128, SUBS, TOPK)),
            )
            ex4 = spool.tile([128, SUBS, TOPK], F32, tag="ex4")
            nc.scalar.activation(
                ex4[:], sh4[:], mybir.ActivationFunctionType.Exp
            )
            sums4 = spool.tile([128, SUBS, 1], F32, tag="sums4")
            nc.vector.tensor_reduce(
                sums4[:, :, 0],
                ex4[:],
                mybir.AxisListType.X,
                mybir.AluOpType.add,
            )
            rs4 = spool.tile([128, SUBS, 1], F32, tag="rs4")
            nc.vector.reciprocal(rs4[:], sums4[:])
            sc4 = spool.tile([128, SUBS, TOPK], F32, tag="sc4")
            nc.vector.tensor_mul(
                sc4[:], ex4[:], rs4[:].broadcast_to((128, SUBS, TOPK))
            )
            pend[c] = (sc4, idx4)

        for step in range(n_chunk + DELTA):
            if step < n_chunk:
                issue_loads(step)
            if step >= DELTA:
                c = step - DELTA
                compute(c)
                # output DMAs lag one chunk so they never gate the next
                # x-load issue in gpsimd's stream
                if c > 0:
                    flush_out(c - 1)
        flush_out(n_chunk - 1)

    nc.compile()
    return nc


_PROGRAM = None


def _get_program():
    global _PROGRAM
    if _PROGRAM is None:
        _PROGRAM = build_program()
    return _PROGRAM


def _make_in_maps(x, weights):
    x = np.asarray(x, dtype=np.float32)
    w = np.asarray(weights, dtype=np.float32)
    maps = []
    for i in range(N_CORES):
        xs = np.ascontiguousarray(x[i * TOK_PER_CORE : (i + 1) * TOK_PER_CORE].T)
        xh = xs.astype(np.float16)
        xl = (xs - xh.astype(np.float32)).astype(np.float16)

        def pack(a):
            # [1024, 8192] -> [chunk, half, 128p, 4k, 512t] contiguous
            return np.ascontiguousarray(
                a.reshape(2, HKCH, 128, N_CHUNK, CHUNK).transpose(3, 0, 2, 1, 4)
            ).reshape(N_CHUNK, 2, 128, HKCH * CHUNK)

        maps.append({"xh": pack(xh), "xl": pack(xl), "w": w})
    return maps


def run(x, weights, trace=False):
    nc = _get_program()
    res = run_bass_kernel_spmd(
        nc, _make_in_maps(x, weights), list(range(N_CORES)), trace=trace
    )

    def unblock(a):
        # [n_chunk, 128, SUBS, TOPK] -> [tok_per_core, TOPK]
        return np.ascontiguousarray(a.transpose(0, 2, 1, 3)).reshape(-1, TOPK)

    scores = np.concatenate(
        [unblock(res.results[i]["scores"]) for i in range(N_CORES)]
    )
    experts = np.concatenate(
        [unblock(res.results[i]["experts"]).astype(np.int32) for i in range(N_CORES)]
    )
    return (scores, experts), res


def kernel(x, weights):
    out, _ = run(x, weights)
    return out


# revision 32
# speedup vs baseline: 1.0888x; 1.0141x over previous
"""MoE gate kernel for TRN2: logits = x @ w, top-8 over 64 experts, softmax.

Sharding: x [65536, 1024] split by token across 8 cores (8192 tokens each),
fed pre-transposed + pre-tiled. w [1024, 64] replicated.

Precision: x shipped as exact fp16 hi/lo pair (x == xh + xl + O(2^-22)),
w split into fp16 hi/lo on device (same DVE conversions as the proven
kernel, so wh/wl are bit-identical to it). The two w halves are packed
side by side into one [128, 128] stationary [wh | wl], so ONE moving
pass of xh computes xh@wh (PSUM partitions 0-63, "B") AND xh@wl
(partitions 64-127, "W") simultaneously; the xl pass adds xl@wh / xl@wl.
16 matmuls/chunk instead of the 3-pass kernel's 24 — B's partial order
(xh@wh k0..7, xl@wh k0..7) is bit-identical to that kernel's first two
passes, and W accumulates at magnitude ~2^-11 so it is error-free.

Top-8 robustness: a plain fl(B+W) comparison carries +-ulp(34) ~ 2e-6
noise, which mis-orders a sub-ULP logit pair in the eval set.  Instead
the logit is kept as an exact Dekker pair: s = fl(B+W), err = W-(s-B)
(TwoSum; s+err == B+W exactly), and the selection key is
r = fl(fl(s - m) + err) with m = per-token max(s).  s-m is exact for
all top-8 contenders (Sterbenz: s >= m/2), so r orders contenders with
error <= ulp(~10)/2 ~ 2.4e-7, an order of magnitude below every
remaining decision margin.  Softmax is shift-invariant, so feeding it
the r-values directly yields the same scores.

DMA: only sync/scalar/gpsimd can issue DMAs, and each queue tops out
around 125-150 GB/s, so the 33.5MB of x per core is striped across all
three, weighted 3:3:2 (gpsimd's software-dynamic queue is the slowest).
Host pre-packs x as [chunk][half][128p][4k][512t] so each 512KB half-
chunk start is contiguous in DRAM; the first two chunks go as 256KB
quarters to shorten the ramp.  The per-chunk loop is software-pipelined
(issue loads for chunk c, compute chunk c-4) so compute ops never block
later x-load issues in an engine's instruction stream; per-chunk output
DMAs are lagged one chunk for the same reason.
"""

import sys

sys.path.insert(0, "/opt/trn_rl_repo")

from contextlib import ExitStack

import numpy as np

import concourse.bacc as bacc
import concourse.mybir as mybir
import concourse.tile as tile
from concourse import masks
from concourse.bass_utils import run_bass_kernel_spmd

N_CORES = 8
TOKENS = 65536
D = 1024
E = 64
TOPK = 8
TOK_PER_CORE = TOKENS // N_CORES
CHUNK = 512  # tokens per chunk (PSUM bank = 512 f32)
SUBS = CHUNK // 128
KCH = D // 128  # contraction chunks
N_CHUNK = TOK_PER_CORE // CHUNK
HKCH = KCH // 2  # k-chunks per DMA half
DELTA = 5  # software pipeline depth (chunks)

F32 = mybir.dt.float32
F16 = mybir.dt.float16
U32 = mybir.dt.uint32


def build_program(tok_per_core=TOK_PER_CORE):
    n_chunk = tok_per_core // CHUNK
    nc = bacc.Bacc(
        "TRN2", target_bir_lowering=False, debug=False, num_devices=N_CORES
    )
    xh_d = nc.dram_tensor(
        "xh", [n_chunk, 2, 128, HKCH * CHUNK], F16, kind="ExternalInput"
    ).ap()
    xl_d = nc.dram_tensor(
        "xl", [n_chunk, 2, 128, HKCH * CHUNK], F16, kind="ExternalInput"
    ).ap()
    w_d = nc.dram_tensor("w", [D, E], F32, kind="ExternalInput").ap()
    scores_d = nc.dram_tensor(
        "scores", [n_chunk, 128, SUBS, TOPK], F32, kind="ExternalOutput"
    ).ap()
    experts_d = nc.dram_tensor(
        "experts", [n_chunk, 128, SUBS, TOPK], U32, kind="ExternalOutput"
    ).ap()

    with tile.TileContext(nc) as tc, ExitStack() as ctx:
        wpool = ctx.enter_context(tc.tile_pool(name="wpool", bufs=1))
        xpool = ctx.enter_context(tc.tile_pool(name="xpool", bufs=8))
        ltpool = ctx.enter_context(tc.tile_pool(name="ltpool", bufs=3))
        ptpool = ctx.enter_context(tc.tile_pool(name="ptpool", bufs=4, space="PSUM"))
        ppool = ctx.enter_context(tc.tile_pool(name="ppool", bufs=3, space="PSUM"))
        spool = ctx.enter_context(tc.tile_pool(name="spool", bufs=4))

        ident = wpool.tile([128, 128], F32)
        masks.make_identity(nc, ident[:])

        # w split on DEVICE with the same DVE fp32->fp16 conversions as
        # the proven kernel; packed side by side: whl = [wh | wl]
        w_t = wpool.tile([128, KCH, E], F32)
        nc.gpsimd.dma_start(
            out=w_t[:], in_=w_d.rearrange("(k p) e -> p k e", p=128)
        )
        whl = wpool.tile([128, KCH, 2 * E], F16)
        nc.vector.tensor_copy(whl[:, :, 0:E], w_t[:])
        wl32 = wpool.tile([128, KCH, E], F32)
        nc.vector.tensor_sub(wl32[:], w_t[:], whl[:, :, 0:E])
        nc.vector.tensor_copy(whl[:, :, E : 2 * E], wl32[:])

        # weighted striping: per 8 pieces, 3 to sync, 3 to scalar, 2 to
        # gpsimd (its software-dynamic queue is ~20% slower, and this
        # pattern never hands gpsimd two pieces of the same chunk — an
        # equal i%3 rotation does, spiking chunk latency and stalling
        # the PE)
        qengines = [nc.sync, nc.scalar, nc.gpsimd]
        qorder = [0, 1, 2, 0, 1, 0, 1, 2]
        piece_i = [0]

        def next_q():
            q = qengines[qorder[piece_i[0] % 8]]
            piece_i[0] += 1
            return q

        xtiles = {}

        def issue_loads(c):
            xh_t = xpool.tile([128, KCH, CHUNK], F16, tag="xh_t")
            xl_t = xpool.tile([128, KCH, CHUNK], F16, tag="xl_t")
            xtiles[c] = (xh_t, xl_t)
            halves = [
                (xh_t[:, 0:HKCH, :], xh_d[c, 0]),
                (xh_t[:, HKCH:KCH, :], xh_d[c, 1]),
                (xl_t[:, 0:HKCH, :], xl_d[c, 0]),
                (xl_t[:, HKCH:KCH, :], xl_d[c, 1]),
            ]
            if c < 2:
                # ramp: quarter-size pieces spread the first chunks across
                # all three queues (256KB pieces for ALL chunks were tried
                # and regress ~5us: the extra issue overhead outweighs any
                # per-queue rate gain)
                Q = HKCH * CHUNK // 2
                for dst, src in halves:
                    next_q().dma_start(out=dst[:, 0 : HKCH // 2, :], in_=src[:, 0:Q])
                    next_q().dma_start(out=dst[:, HKCH // 2 : HKCH, :], in_=src[:, Q:])
            else:
                for dst, src in halves:
                    next_q().dma_start(out=dst, in_=src)

        pend = {}

        def flush_out(c):
            sc4, idx4 = pend.pop(c)
            nc.gpsimd.dma_start(out=scores_d[c], in_=sc4[:])
            nc.gpsimd.dma_start(out=experts_d[c], in_=idx4[:])

        def compute(c):
            xh_t, xl_t = xtiles.pop(c)

            # 16-matmul chain; partitions 0-63 see xh@wh k0..7 then
            # xl@wh k0..7 — bit-identical to the proven kernel's first
            # 16 partials
            lps = ptpool.tile([128, CHUNK], F32)
            for k in range(KCH):
                nc.tensor.matmul(
                    lps[:], whl[:, k, :], xh_t[:, k, :],
                    start=(k == 0), stop=False,
                )
            for k in range(KCH):
                nc.tensor.matmul(
                    lps[:], whl[:, k, :], xl_t[:, k, :],
                    start=False, stop=(k == KCH - 1),
                )

            # PSUM -> SBUF on the scalar engine (frees DVE)
            ltile = ltpool.tile([128, CHUNK], F32)
            nc.scalar.activation(
                ltile[:], lps[:], mybir.ActivationFunctionType.Copy
            )

            # 4 transposes into one PSUM bank -> [tok, s, B-col | W-col]
            tpb = ppool.tile([128, SUBS, 128], F32)
            for s in range(SUBS):
                nc.tensor.transpose(
                    tpb[:, s, :], ltile[:, s * 128 : (s + 1) * 128], ident[:]
                )
            # stage the W half through SBUF (ops may read only one PSUM
            # operand, and gpsimd none)
            wlp4 = spool.tile([128, SUBS, E], F32, tag="wlp4")
            nc.scalar.activation(
                wlp4[:], tpb[:, :, E : 2 * E],
                mybir.ActivationFunctionType.Copy,
            )

            # Dekker TwoSum: s4 + er4 == B + W exactly
            s4 = spool.tile([128, SUBS, E], F32, tag="s4")
            nc.vector.tensor_add(s4[:], tpb[:, :, 0:E], wlp4[:])
            bb4 = spool.tile([128, SUBS, E], F32, tag="bb4")
            nc.vector.tensor_sub(bb4[:], s4[:], tpb[:, :, 0:E])
            er4 = spool.tile([128, SUBS, E], F32, tag="er4")
            nc.vector.tensor_sub(er4[:], wlp4[:], bb4[:])
            # refined selection key r = (s - max) + err (exact shift for
            # contenders, then one rounding at ~ulp(10))
            m4 = spool.tile([128, SUBS, 1], F32, tag="m4")
            nc.vector.tensor_reduce(
                m4[:, :, 0], s4[:], mybir.AxisListType.X, mybir.AluOpType.max
            )
            d4 = spool.tile([128, SUBS, E], F32, tag="d4")
            nc.vector.tensor_sub(
                d4[:], s4[:], m4[:].broadcast_to((128, SUBS, E))
            )
            r4 = spool.tile([128, SUBS, E], F32, tag="r4")
            nc.vector.tensor_add(r4[:], d4[:], er4[:])

            vals4 = spool.tile([128, SUBS, TOPK], F32, tag="vals4")
            idx4 = spool.tile([128, SUBS, TOPK], U32, tag="idx4")
            for s in range(SUBS):
                nc.vector.max(vals4[:, s, :], r4[:, s, :])
                nc.vector.max_index(idx4[:, s, :], vals4[:, s, :], r4[:, s, :])

            # batched softmax over the 4 sub-tiles: exp(v - max) / sum
            # (r is the logit shifted per token, so scores are unchanged)
            sh4 = spool.tile([128, SUBS, TOPK], F32, tag="sh4")
            nc.vector.tensor_sub(
                sh4[:],
                vals4[:],
                vals4[:, :, 0:1].broadcast_to((128, SUBS, TOPK)),
            )
            ex4 = spool.tile([128, SUBS, TOPK], F32, tag="ex4")
            nc.scalar.activation(
                ex4[:], sh4[:], mybir.ActivationFunctionType.Exp
            )
            sums4 = spool.tile([128, SUBS, 1], F32, tag="sums4")
            nc.vector.tensor_reduce(
                sums4[:, :, 0],
                ex4[:],
                mybir.AxisListType.X,
                mybir.AluOpType.add,
            )
            rs4 = spool.tile([128, SUBS, 1], F32, tag="rs4")
            nc.vector.reciprocal(rs4[:], sums4[:])
            sc4 = spool.tile([128, SUBS, TOPK], F32, tag="sc4")
            nc.vector.tensor_mul(
                sc4[:], ex4[:], rs4[:].broadcast_to((128, SUBS, TOPK))
            )
            pend[c] = (sc4, idx4)

        for step in range(n_chunk + DELTA):
            if step < n_chunk:
                issue_loads(step)
            if step >= DELTA:
                c = step - DELTA
                compute(c)
                # output DMAs lag one chunk so they never gate the next
                # x-load issue in gpsimd's stream
                if c > 0:
                    flush_out(c - 1)
        flush_out(n_chunk - 1)

    nc.compile()
    return nc


_PROGRAM = None


def _get_program():
    global _PROGRAM
    if _PROGRAM is None:
        _PROGRAM = build_program()
    return _PROGRAM


def _make_in_maps(x, weights):
    x = np.asarray(x, dtype=np.float32)
    w = np.asarray(weights, dtype=np.float32)
    maps = []
    for i in range(N_CORES):
        xs = np.ascontiguousarray(x[i * TOK_PER_CORE : (i + 1) * TOK_PER_CORE].T)
        xh = xs.astype(np.float16)
        xl = (xs - xh.astype(np.float32)).astype(np.float16)

        def pack(a):
            # [1024, 8192] -> [chunk, half, 128p, 4k, 512t] contiguous
            return np.ascontiguousarray(
                a.reshape(2, HKCH, 128, N_CHUNK, CHUNK).transpose(3, 0, 2, 1, 4)
            ).reshape(N_CHUNK, 2, 128, HKCH * CHUNK)

        maps.append({"xh": pack(xh), "xl": pack(xl), "w": w})
    return maps


def run(x, weights, trace=False):
    nc = _get_program()
    res = run_bass_kernel_spmd(
        nc, _make_in_maps(x, weights), list(range(N_CORES)), trace=trace
    )

    def unblock(a):
        # [n_chunk, 128, SUBS, TOPK] -> [tok_per_core, TOPK]
        return np.ascontiguousarray(a.transpose(0, 2, 1, 3)).reshape(-1, TOPK)

    scores = np.concatenate(
        [unblock(res.results[i]["scores"]) for i in range(N_CORES)]
    )
    experts = np.concatenate(
        [unblock(res.results[i]["experts"]).astype(np.int32) for i in range(N_CORES)]
    )
    return (scores, experts), res


def kernel(x, weights):
    out, _ = run(x, weights)
    return out
